# revision 9
# baseline (speedup 1.0000x reference)
"""Trainium2 Bass kernel for nn_Attention_609885356930.

Reference math (per batch b, sequence s):
    term1[b,s,k] = sum_d WO[k,d] * x[b,s,d]          # big matmul
    term2[b,k]   = sum_d WG[k,d] * g[b,d]            # tiny matmul
    out[b,s]     = sum_k v[k] * tanh(term1 + term2)

Strategy (8 NeuronCores, data-parallel over batch, 4 batches/core):
  - Host pre-packs every DRAM input into the exact SBUF image layout
    (contraction dim d on partitions, contiguous per-partition segments)
    so each DMA is a full-rate sequential read.
  - Compute term1 transposed on-chip: T1[k_block, s] so that
      * term2 becomes a per-partition bias fused into the ACT tanh pass
      * the v-weighted reduce over k runs on the otherwise-idle DVE as
        per-partition-scalar multiply-accumulates, finished by a single
        ones-vector PE matmul per s-block (partition reduction).
  - bf16 matmuls (rel-err budget 2e-2), fp32 PSUM accumulation.
  - Startup: WG sliced per k-block so term2 starts after ~256 KB of DMA;
    dummy matmuls keep the PE busy (HAM un-throttled) during the preload.
"""

import numpy as np
import ml_dtypes
from contextlib import ExitStack

import concourse.bass as bass
import concourse.mybir as mybir
import concourse.tile as tile
from concourse import bacc
from concourse.bass_utils import run_bass_kernel_spmd

B, S, D, K = 32, 2048, 1024, 1024
NCORES = 8
LB = B // NCORES          # local batches per core
P = 128                   # SBUF partitions
NCH = D // P              # contraction chunks (8)
NM = K // P               # output k-blocks (8)
SBLK = 512                # s-tile width (one PSUM bank of fp32)
NSBLK = S // SBLK

BF16 = mybir.dt.bfloat16
F32 = mybir.dt.float32
Tanh = mybir.ActivationFunctionType.Tanh


def build(lb=LB, s=S, d=D, k=K, sblk=SBLK):
    nch = d // P
    nm = k // P
    nsblk = s // sblk

    nc = bacc.Bacc("TRN2", target_bir_lowering=False, debug=False)
    # All inputs pre-packed host-side into SBUF-image layouts:
    xt_d = nc.declare_dram_parameter("xt", [lb, nsblk, P, nch, sblk], BF16,
                                     isOutput=False)
    wot_d = nc.declare_dram_parameter("wot", [nch, P, k], BF16, isOutput=False)
    wgt_d = nc.declare_dram_parameter("wgt", [nm, P, nch, P], BF16, isOutput=False)
    gt_d = nc.declare_dram_parameter("gt", [P, nch, lb], BF16, isOutput=False)
    v_d = nc.declare_dram_parameter("v", [P, nm], F32, isOutput=False)
    out_d = nc.declare_dram_parameter("out", [lb, s], F32, isOutput=True)

    with ExitStack() as ctx:
        tc = ctx.enter_context(tile.TileContext(nc))
        const = ctx.enter_context(tc.tile_pool(name="const", bufs=1))
        xpool = ctx.enter_context(tc.tile_pool(name="xpool", bufs=3))
        tpool = ctx.enter_context(tc.tile_pool(name="tpool", bufs=3))
        apool = ctx.enter_context(tc.tile_pool(name="apool", bufs=2))
        opool = ctx.enter_context(tc.tile_pool(name="opool", bufs=2))
        ppool = ctx.enter_context(tc.tile_pool(name="ppool", bufs=3, space="PSUM"))
        popool = ctx.enter_context(tc.tile_pool(name="popool", bufs=2, space="PSUM"))

        # ---- constants: WG first (term2 is on the PE critical path),
        # sliced per k-block so term2(m) starts after ~256 KB of DMA ----
        g_sb = const.tile([P, nch, lb], BF16)
        nc.sync.dma_start(g_sb[:], gt_d[:])
        v_sb = const.tile([P, nm], F32)
        nc.sync.dma_start(v_sb[:], v_d[:])
        ones_sb = const.tile([P, 1], F32)
        nc.vector.memset(ones_sb[:], 1.0)

        wg_sb = const.tile([P, nch, k], BF16)
        term2_sb = const.tile([P, nm * lb], F32)
        for m in range(nm):
            ksl = slice(m * P, (m + 1) * P)
            nc.sync.dma_start(wg_sb[:, :, ksl], wgt_d[m])
            ps_t2 = ppool.tile([P, lb], F32, tag="pst2", bufs=1)
            for c in range(nch):
                nc.tensor.matmul(
                    ps_t2[:],
                    wg_sb[:, c, ksl],
                    g_sb[:, c, :],
                    start=(c == 0),
                    stop=(c == nch - 1),
                )
            nc.vector.tensor_copy(term2_sb[:, m * lb:(m + 1) * lb], ps_t2[:])
            # dummy matmuls on already-loaded data: keep the PE busy through
            # the startup DMA so HAM un-throttles before the real stream
            if m < 6:
                ps_w = ppool.tile([P, sblk], F32, tag="psw", bufs=1)
                for _ in range(3):
                    nc.tensor.matmul(
                        ps_w[:],
                        wg_sb[:, 0, 0:P],
                        wg_sb[:, 0:sblk // P, 0:P],
                        start=True,
                        stop=True,
                    )

        # WO streamed per-chunk, interleaved with the first x tile's chunks so
        # the first main matmul group starts as soon as chunk 0 arrives.
        wo_sb = const.tile([P, nch, k], BF16)
        xt0_sb = xpool.tile([P, nch, sblk], BF16, tag="xt")
        for c in range(nch):
            nc.sync.dma_start(wo_sb[:, c, :], wot_d[c])
            nc.sync.dma_start(xt0_sb[:, c, :], xt_d[0, 0, :, c, :])

        # ---- main loop ----
        for b in range(lb):
            orow = opool.tile([1, s], F32, tag="orow")
            for i in range(nsblk):
                s0 = i * sblk
                if b == 0 and i == 0:
                    xt_sb = xt0_sb
                else:
                    xt_sb = xpool.tile([P, nch, sblk], BF16, tag="xt")
                    nc.sync.dma_start(xt_sb[:], xt_d[b, i])
                acc = apool.tile([P, sblk], F32, tag="acc")
                tmp = apool.tile([P, sblk], F32, tag="tmp")
                for m in range(nm):
                    ps1 = ppool.tile([P, sblk], F32, tag="ps1")
                    for c in range(nch):
                        nc.tensor.matmul(
                            ps1[:],
                            wo_sb[:, c, m * P:(m + 1) * P],
                            xt_sb[:, c, :],
                            start=(c == 0),
                            stop=(c == nch - 1),
                        )
                    th = tpool.tile([P, sblk], BF16, tag="th")
                    nc.scalar.activation(
                        th[:], ps1[:], Tanh,
                        bias=term2_sb[:, m * lb + b:m * lb + b + 1],
                    )
                    # v-weighted accumulate over k-blocks on DVE
                    if m == 0:
                        nc.vector.tensor_scalar_mul(acc[:], th[:], v_sb[:, 0:1])
                    else:
                        nc.vector.tensor_scalar_mul(tmp[:], th[:], v_sb[:, m:m + 1])
                        nc.vector.tensor_add(out=acc[:], in0=acc[:], in1=tmp[:])
                # partition reduction: out_row[s] = sum_p acc[p, s]
                ps_o = popool.tile([1, sblk], F32, tag="pso")
                nc.tensor.matmul(
                    ps_o[:], ones_sb[:], acc[:], start=True, stop=True,
                )
                nc.vector.tensor_copy(orow[0:1, s0:s0 + sblk], ps_o[:])
            nc.sync.dma_start(out_d[b:b + 1, :], orow[:])
    nc.compile()
    return nc


def pack_inputs(x, g, WO, WG, v, lb, s=S, d=D, k=K, sblk=SBLK):
    """Pack one core's inputs into the DRAM image layouts declared in build()."""
    bf16 = ml_dtypes.bfloat16
    nch = d // P
    nm = k // P
    nsblk = s // sblk
    # xt[b, i, p, c, s_in] = x[b, i*sblk + s_in, c*P + p]
    xt = np.ascontiguousarray(
        x.reshape(lb, nsblk, sblk, nch, P).transpose(0, 1, 4, 3, 2)
    ).astype(bf16)
    # wot[c, p, k] = WO[k, c*P + p]
    wot = np.ascontiguousarray(WO.T.reshape(nch, P, k)).astype(bf16)
    # wgt[m, p, c, k_in] = WG[m*P + k_in, c*P + p]
    wgt = np.ascontiguousarray(
        WG.T.reshape(nch, P, nm, P).transpose(2, 1, 0, 3)
    ).astype(bf16)
    # gt[p, c, b] = g[b, c*P + p]
    gt = np.ascontiguousarray(g.T.reshape(nch, P, lb).transpose(1, 0, 2)).astype(bf16)
    # v[p, m] = v[m*P + p]
    vi = np.ascontiguousarray(v.reshape(nm, P).T).astype(np.float32)
    return {"xt": xt, "wot": wot, "wgt": wgt, "gt": gt, "v": vi}


_built = None


def _get_built():
    global _built
    if _built is None:
        _built = build()
    return _built


def make_in_maps(inputs_np):
    x = np.asarray(inputs_np["inputs"], dtype=np.float32)
    g = np.asarray(inputs_np["g"], dtype=np.float32)
    WO = np.asarray(inputs_np["WO"], dtype=np.float32)
    WG = np.asarray(inputs_np["WG"], dtype=np.float32)
    v = np.asarray(inputs_np["v"], dtype=np.float32)[0]

    shared = None
    in_maps = []
    for i in range(NCORES):
        m = pack_inputs(x[i * LB:(i + 1) * LB], g[i * LB:(i + 1) * LB],
                        WO, WG, v, lb=LB)
        if shared is None:
            shared = {kk: m[kk] for kk in ("wot", "wgt", "v")}
        else:
            m.update(shared)  # identical weight images for every core
        in_maps.append(m)
    return in_maps


def run(inputs_np, trace=False):
    nc = _get_built()
    in_maps = make_in_maps(inputs_np)
    res = run_bass_kernel_spmd(nc, in_maps, core_ids=list(range(NCORES)), trace=trace)
    out = np.concatenate(
        [np.asarray(res.results[i]["out"]) for i in range(NCORES)], axis=0
    ).astype(np.float32)
    return out, res


def kernel(**inputs):
    out, _ = run(inputs, trace=False)
    return out


# revision 10
# speedup vs baseline: 1.1700x; 1.1700x over previous
"""Trainium2 Bass kernel for nn_Attention_609885356930.

Reference math (per batch b, sequence s):
    term1[b,s,k] = sum_d WO[k,d] * x[b,s,d]          # big matmul
    term2[b,k]   = sum_d WG[k,d] * g[b,d]            # tiny matmul
    out[b,s]     = sum_k v[k] * tanh(term1 + term2)

Strategy (8 NeuronCores, data-parallel over batch, 4 batches/core):
  - Host pre-transposes x -> xT[b, d, s] and weights -> WO.T / WG.T (bf16),
    so the contraction dim d lands on SBUF partitions with no on-device
    transpose.
  - Compute term1 transposed on-chip: T1[k_block, s] so that
      * term2 becomes a per-partition bias fused into the ACT tanh pass
      * the v-weighted reduce over k runs on the otherwise-idle DVE as
        per-partition-scalar multiply-accumulates, finished by a single
        ones-vector PE matmul per s-block (partition reduction).
  - bf16 matmuls (rel-err budget 2e-2), fp32 PSUM accumulation.
  - Startup: memset-fed dummy matmuls warm the PE (HAM) with no DMA
    dependency; WG loads on the scalar HWDGE queue while WO/x stream on
    the sync queue.
"""

import numpy as np
import ml_dtypes
from contextlib import ExitStack

import concourse.bass as bass
import concourse.mybir as mybir
import concourse.tile as tile
from concourse import bacc
from concourse.bass_utils import run_bass_kernel_spmd

B, S, D, K = 32, 2048, 1024, 1024
NCORES = 8
LB = B // NCORES          # local batches per core
P = 128                   # SBUF partitions
NCH = D // P              # contraction chunks (8)
NM = K // P               # output k-blocks (8)
SBLK = 512                # s-tile width (one PSUM bank of fp32)

BF16 = mybir.dt.bfloat16
F32 = mybir.dt.float32
Tanh = mybir.ActivationFunctionType.Tanh


def build(lb=LB, s=S, d=D, k=K, sblk=SBLK, n_warm=20):
    nch = d // P
    nm = k // P
    nsblk = s // sblk

    nc = bacc.Bacc("TRN2", target_bir_lowering=False, debug=False)
    xt_d = nc.declare_dram_parameter("xt", [lb, d, s], BF16, isOutput=False)
    wot_d = nc.declare_dram_parameter("wot", [d, k], BF16, isOutput=False)
    wgt_d = nc.declare_dram_parameter("wgt", [d, k], BF16, isOutput=False)
    gt_d = nc.declare_dram_parameter("gt", [d, lb], BF16, isOutput=False)
    v_d = nc.declare_dram_parameter("v", [k], F32, isOutput=False)
    out_d = nc.declare_dram_parameter("out", [lb, s], F32, isOutput=True)

    with ExitStack() as ctx:
        tc = ctx.enter_context(tile.TileContext(nc))
        const = ctx.enter_context(tc.tile_pool(name="const", bufs=1))
        xpool = ctx.enter_context(tc.tile_pool(name="xpool", bufs=3))
        tpool = ctx.enter_context(tc.tile_pool(name="tpool", bufs=3))
        apool = ctx.enter_context(tc.tile_pool(name="apool", bufs=2))
        opool = ctx.enter_context(tc.tile_pool(name="opool", bufs=2))
        ppool = ctx.enter_context(tc.tile_pool(name="ppool", bufs=3, space="PSUM"))
        popool = ctx.enter_context(tc.tile_pool(name="popool", bufs=2, space="PSUM"))

        # ---- PE warm-up: dummy matmuls fed from a memset tile (no DMA dep)
        # keep the PE busy from t~0 so HAM un-throttles before real work ----
        warm_sb = const.tile([P, P + sblk], BF16)
        nc.vector.memset(warm_sb[:], 0.25)
        ps_w = ppool.tile([P, sblk], F32, tag="psw", bufs=1)
        for _ in range(n_warm):
            nc.tensor.matmul(
                ps_w[:], warm_sb[:, 0:P], warm_sb[:, P:P + sblk],
                start=True, stop=True,
            )

        # ---- constants: WG on the scalar HWDGE queue (term2 critical path),
        # WO/x stream on the sync queue concurrently ----
        wg_sb = const.tile([P, nch, k], BF16)
        nc.scalar.dma_start(wg_sb[:], wgt_d.rearrange("(c p) k -> p c k", p=P))
        g_sb = const.tile([P, nch, lb], BF16)
        nc.scalar.dma_start(g_sb[:], gt_d.rearrange("(c p) b -> p c b", p=P))
        v_sb = const.tile([P, nm], F32)
        nc.scalar.dma_start(v_sb[:], v_d.rearrange("(m p) -> p m", p=P))
        ones_sb = const.tile([P, 1], F32)
        nc.vector.memset(ones_sb[:], 1.0)

        # WO + first x tile interleaved per-chunk on sync queue
        wo_sb = const.tile([P, nch, k], BF16)
        xt0_sb = xpool.tile([P, nch, sblk], BF16, tag="xt")
        xt0_src = xt_d[0].rearrange("(c p) s -> p c s", p=P)[:, :, 0:sblk]
        wot_src = wot_d.rearrange("(c p) k -> p c k", p=P)
        for c in range(nch):
            nc.sync.dma_start(wo_sb[:, c, :], wot_src[:, c, :])
            nc.sync.dma_start(xt0_sb[:, c, :], xt0_src[:, c, :])

        # term2[k, b] for all local batches: [128, nm * lb] fp32
        term2_sb = const.tile([P, nm * lb], F32)
        for m in range(nm):
            ps_t2 = ppool.tile([P, lb], F32, tag="pst2", bufs=1)
            for c in range(nch):
                nc.tensor.matmul(
                    ps_t2[:],
                    wg_sb[:, c, m * P:(m + 1) * P],
                    g_sb[:, c, :],
                    start=(c == 0),
                    stop=(c == nch - 1),
                )
            nc.vector.tensor_copy(term2_sb[:, m * lb:(m + 1) * lb], ps_t2[:])

        # ---- main loop ----
        for b in range(lb):
            orow = opool.tile([1, s], F32, tag="orow")
            for i in range(nsblk):
                s0 = i * sblk
                if b == 0 and i == 0:
                    xt_sb = xt0_sb
                else:
                    xt_sb = xpool.tile([P, nch, sblk], BF16, tag="xt")
                    nc.sync.dma_start(
                        xt_sb[:],
                        xt_d[b].rearrange("(c p) s -> p c s", p=P)[:, :, s0:s0 + sblk],
                    )
                acc = apool.tile([P, sblk], F32, tag="acc")
                tmp = apool.tile([P, sblk], F32, tag="tmp")
                for m in range(nm):
                    ps1 = ppool.tile([P, sblk], F32, tag="ps1")
                    for c in range(nch):
                        nc.tensor.matmul(
                            ps1[:],
                            wo_sb[:, c, m * P:(m + 1) * P],
                            xt_sb[:, c, :],
                            start=(c == 0),
                            stop=(c == nch - 1),
                        )
                    th = tpool.tile([P, sblk], BF16, tag="th")
                    nc.scalar.activation(
                        th[:], ps1[:], Tanh,
                        bias=term2_sb[:, m * lb + b:m * lb + b + 1],
                    )
                    # v-weighted accumulate over k-blocks on DVE
                    if m == 0:
                        nc.vector.tensor_scalar_mul(acc[:], th[:], v_sb[:, 0:1])
                    else:
                        nc.vector.tensor_scalar_mul(tmp[:], th[:], v_sb[:, m:m + 1])
                        nc.vector.tensor_add(out=acc[:], in0=acc[:], in1=tmp[:])
                # partition reduction: out_row[s] = sum_p acc[p, s]
                ps_o = popool.tile([1, sblk], F32, tag="pso")
                nc.tensor.matmul(
                    ps_o[:], ones_sb[:], acc[:], start=True, stop=True,
                )
                nc.vector.tensor_copy(orow[0:1, s0:s0 + sblk], ps_o[:])
            nc.sync.dma_start(out_d[b:b + 1, :], orow[:])
    nc.compile()
    return nc


def pack_inputs(x, g, WO, WG, v, lb, s=S, d=D, k=K, sblk=SBLK):
    """Pack one core's inputs into the DRAM layouts declared in build()."""
    bf16 = ml_dtypes.bfloat16
    xt = np.ascontiguousarray(x.transpose(0, 2, 1)).astype(bf16)        # [lb, d, s]
    wot = np.ascontiguousarray(WO.T).astype(bf16)                       # [d, k]
    wgt = np.ascontiguousarray(WG.T).astype(bf16)                       # [d, k]
    gt = np.ascontiguousarray(g.T).astype(bf16)                         # [d, lb]
    vi = np.ascontiguousarray(v).astype(np.float32)                     # [k]
    return {"xt": xt, "wot": wot, "wgt": wgt, "gt": gt, "v": vi}


_built = None


def _get_built():
    global _built
    if _built is None:
        _built = build()
    return _built


def make_in_maps(inputs_np):
    x = np.asarray(inputs_np["inputs"], dtype=np.float32)
    g = np.asarray(inputs_np["g"], dtype=np.float32)
    WO = np.asarray(inputs_np["WO"], dtype=np.float32)
    WG = np.asarray(inputs_np["WG"], dtype=np.float32)
    v = np.asarray(inputs_np["v"], dtype=np.float32)[0]

    shared = None
    in_maps = []
    for i in range(NCORES):
        m = pack_inputs(x[i * LB:(i + 1) * LB], g[i * LB:(i + 1) * LB],
                        WO, WG, v, lb=LB)
        if shared is None:
            shared = {kk: m[kk] for kk in ("wot", "wgt", "v")}
        else:
            m.update(shared)  # identical weight images for every core
        in_maps.append(m)
    return in_maps


def run(inputs_np, trace=False):
    nc = _get_built()
    in_maps = make_in_maps(inputs_np)
    res = run_bass_kernel_spmd(nc, in_maps, core_ids=list(range(NCORES)), trace=trace)
    out = np.concatenate(
        [np.asarray(res.results[i]["out"]) for i in range(NCORES)], axis=0
    ).astype(np.float32)
    return out, res


def kernel(**inputs):
    out, _ = run(inputs, trace=False)
    return out
